# revision 1
# baseline (speedup 1.0000x reference)
"""Trainium2 Bass kernel: fp8-quantized Dense (8192x4096 @ 4096x16384) + bias + tanh-GELU.

Strategy (tensor-parallel over units, 8 cores):
  - host: transpose x -> xT [d_in, tokens]; shard kernel/bias along units.
  - device per core:
      phase 1a: amax scan of this core's kernel shard (DVE abs-max), then
                AllReduce(max) -> global kernel amax (CC_k).
      phase 1b: amax scan of this core's 1/8 column slice of xT (overlaps
                CC_k), then AllReduce(max) -> global x amax (CC_x).
      phase 2:  quantize the kernel shard to fp8e4, resident in SBUF
                (first slabs are prefetched in f32 during the scans and
                quantized as soon as CC_k lands).
      phase 3:  stream xT token-blocks: quantize to fp8e4, DoubleRow fp8
                matmuls accumulating over d_in in PSUM, single-ACT epilogue
                gelu_tanh(psum * inv_scale + bias) per [128,512] tile, DMA out.
  - fp8 numerics: the reference quantizes with scale 448/amax onto the OCP
    e4m3fn grid. TRN fp8e4 tops out at 240 but matches e4m3fn exactly in
    [-240, 240]. Quantizing with 224/amax (= half the reference scale, a
    power-of-two ratio) lands on the identical grid after dequant: the hw
    fp8 values are exactly half the reference's. The dequant scale
    amax_x*amax_k/224^2 restores the reference computation up to f32
    accumulation order.
  - output is produced transposed ([units, tokens] per core); the host
    gathers shards and returns the [tokens, units] transpose view.
"""

import sys

sys.path.insert(0, "/opt/trn_rl_repo")

from contextlib import ExitStack

import numpy as np

import concourse.bacc as bacc
import concourse.tile as tile
from concourse import mybir
from concourse.bass_utils import run_bass_kernel_spmd

P = 128
FP8_HW_MAX = 224.0  # 448/2: keeps hw fp8 values inside TRN's +/-240 range

TOKENS, D_IN, UNITS, N_CORES = 8192, 4096, 16384, 8

KPRE_GROUPS = 2  # k-slab f32 prefetch groups staged in xq-pool slots
KPRE_PER_GROUP = 4


def _blocks(tokens, tblk):
    """Token-block schedule: small warmup blocks so PE starts earlier."""
    if tokens >= 4 * tblk and tblk >= 1024:
        small = tblk // 2
        rest = tokens - 4 * small
        assert rest % tblk == 0
        return [small] * 4 + [tblk] * (rest // tblk)
    assert tokens % tblk == 0
    return [tblk] * (tokens // tblk)


def build(tokens=TOKENS, d_in=D_IN, units=UNITS, n_cores=N_CORES, tblk=1024, nfree=512):
    us = units // n_cores
    ko_n = d_in // P          # 128-row f32 slabs along d_in
    kk_n = d_in // (2 * P)    # DoubleRow (256-contraction) steps
    nu = us // P              # 128-unit output blocks
    amx_t = tokens // n_cores # columns of xT this core amax-scans
    blocks = _blocks(tokens, tblk)

    assert d_in % (2 * P) == 0 and us % P == 0
    assert all(b % nfree == 0 for b in blocks)

    n_groups = min(KPRE_GROUPS, ko_n // KPRE_PER_GROUP)
    n_kpre = n_groups * KPRE_PER_GROUP

    dt = mybir.dt
    f32 = dt.float32
    fp8 = dt.float8e4
    X = mybir.AxisListType.X
    MAX = mybir.AluOpType.max

    nc = bacc.Bacc("TRN2", target_bir_lowering=False, debug=False, num_devices=n_cores)
    xT = nc.dram_tensor("xT", [d_in, tokens], f32, kind="ExternalInput").ap()
    xsl = nc.dram_tensor("xsl", [d_in, amx_t], f32, kind="ExternalInput").ap()
    ksh = nc.dram_tensor("ksh", [d_in, us], f32, kind="ExternalInput").ap()
    bsh = nc.dram_tensor("bsh", [us], f32, kind="ExternalInput").ap()
    out = nc.dram_tensor("out", [us, tokens], f32, kind="ExternalOutput").ap()

    smax = max(us, tblk, amx_t)

    def ldq(i):
        # alternate big loads across two HWDGE DMA queues (sync / scalar)
        return nc.sync if i % 2 == 0 else nc.scalar

    from concourse.tile_rust import add_dep_helper

    with tile.TileContext(nc) as tc, ExitStack() as ctx:
        const = ctx.enter_context(tc.tile_pool(name="const", bufs=1))
        kstage = ctx.enter_context(tc.tile_pool(name="kstage", bufs=3))
        xstage = ctx.enter_context(tc.tile_pool(name="xstage", bufs=5))
        kqp = ctx.enter_context(tc.tile_pool(name="kqp", bufs=1))
        xqp = ctx.enter_context(tc.tile_pool(name="xqp", bufs=2))
        outp = ctx.enter_context(tc.tile_pool(name="outp", bufs=8))
        psum = ctx.enter_context(tc.tile_pool(name="psum", bufs=8, space="PSUM"))
        dram = ctx.enter_context(tc.tile_pool(name="dram", bufs=1, space="DRAM"))
        small = ctx.enter_context(tc.tile_pool(name="small", bufs=1))
        xsmax = max(amx_t, tblk)

        from concourse import bass_isa

        def partition_amax_to(dst, racc, name):
            """[P, ko_n] per-partition maxes -> [1,1] scalar in dst (SBUF)."""
            col = small.tile([P, 1], f32, name=f"{name}_col")
            nc.vector.tensor_reduce(col[:], racc[:], axis=X, op=MAX)
            nc.gpsimd.partition_all_reduce(col[:], col[:], P, bass_isa.ReduceOp.max)
            nc.vector.tensor_copy(dst, col[0:1, :])

        def allreduce_max(src8, name):
            """AllReduce(max) of a [1,8] SBUF tile; returns [1,8] SBUF result."""
            cc_in = dram.tile([1, 8], f32, name=f"{name}_in")
            nc.sync.dma_start(cc_in[:], src8[:])
            cc_out = dram.tile([1, 8], f32, name=f"{name}_out", addr_space="Shared")
            nc.gpsimd.collective_compute(
                "AllReduce", MAX,
                replica_groups=[list(range(n_cores))],
                ins=[cc_in[:].opt()], outs=[cc_out[:].opt()],
            )
            res = small.tile([1, 8], f32, name=f"{name}_res")
            nc.sync.dma_start(res[:], cc_out[:])
            return res

        def bcast_scalar(src11, name):
            """[1,1] SBUF scalar (partition 0) -> [P,1] SBUF broadcast tile."""
            b = const.tile([P, 1], f32, name=f"{name}_b")
            nc.gpsimd.partition_broadcast(b[:], src11)
            return b

        # ---- phase 1a: kernel-shard amax scan (first on the wire) ----
        # The last n_kpre slabs land in the (still idle) xq-pool slots and are
        # RETAINED in f32 until the scale arrives, so they need no re-stream.
        rk_all = const.tile([P, ko_n], f32, name="rk_all")
        n_stream = ko_n - n_kpre
        last_kscan_dma = None
        for ko in range(n_stream):
            st = kstage.tile([P, us], f32, tag="kst", name="amx_k_st")
            last_kscan_dma = ldq(ko).dma_start(st[:], ksh[ko * P : (ko + 1) * P, :])
            nc.vector.tensor_reduce(
                rk_all[:, ko : ko + 1], st[:], axis=X, op=MAX,
                apply_absolute_value=True,
            )
        kret = []
        for g in range(n_groups):
            t = xqp.tile([P, KPRE_PER_GROUP, us], f32, tag="xq", name=f"kret{g}")
            for j in range(KPRE_PER_GROUP):
                ko = n_stream + g * KPRE_PER_GROUP + j
                last_kscan_dma = ldq(ko).dma_start(
                    t[:, j], ksh[ko * P : (ko + 1) * P, :]
                )
                nc.vector.tensor_reduce(
                    rk_all[:, ko : ko + 1], t[:, j], axis=X, op=MAX,
                    apply_absolute_value=True,
                )
            kret.append(t)

        # ---- phase 1b: x-slice amax scan (after the k-scan wire-wise) ----
        rx_all = const.tile([P, ko_n], f32, name="rx_all")
        last_xscan_dma = None
        for ko in range(ko_n):
            st = xstage.tile([P, xsmax], f32, tag="xst", name="amx_x_st")
            last_xscan_dma = ldq(ko).dma_start(
                st[:, :amx_t], xsl[ko * P : (ko + 1) * P, :]
            )
            if ko == 0 and last_kscan_dma is not None:
                add_dep_helper(
                    last_xscan_dma.ins, last_kscan_dma.ins, sync=True,
                    reason="x-scan starts after k-scan",
                )
            nc.vector.tensor_reduce(
                rx_all[:, ko : ko + 1], st[:, :amx_t], axis=X, op=MAX,
                apply_absolute_value=True,
            )

        # ---- single AllReduce(max) of [amax_k, amax_x] ----
        pk8 = small.tile([1, 8], f32, name="pk8")
        nc.vector.memset(pk8[:], 0.0)
        partition_amax_to(pk8[:, 0:1], rk_all, "pk")
        partition_amax_to(pk8[:, 1:2], rx_all, "px")
        g8 = allreduce_max(pk8, "cc")

        d2 = small.tile([1, 2], f32, name="d2")
        nc.vector.tensor_scalar_max(d2[:], g8[:, 0:2], 1e-12)

        # Correctly-rounded s = RNE(224/d): the quantize grid must bit-match the
        # reference's RNE(448/d)/2. DVE has no divide, and reciprocal+multiply
        # is 1-2 ulp off, which flips RNE decisions for ~1e-6 of elements and
        # costs ~2e-3 absmax error. Instead: Newton-refine 224*recip(d) with a
        # Dekker-exact residual (lands within ~0.51 ulp), then pick among 5
        # float-constructed neighbor candidates the one minimizing |q*d - 224|.
        # Verified in numpy over 300k random/binade-edge d with seeds up to
        # +-3 ulp off: 0 mismatches vs IEEE f32 division.
        NCAND = 5
        u32 = dt.uint32
        MUL = mybir.AluOpType.mult
        SUB = mybir.AluOpType.subtract
        ADD = mybir.AluOpType.add

        def tt(out, a, bb, op):
            nc.vector.tensor_tensor(out, a, bb, op)

        def c3(name):
            return small.tile([1, 2, NCAND], f32, name=name)

        def vsplit(src, pref, shape=(1, 2)):
            t_ = small.tile(list(shape), f32, name=f"{pref}_t")
            nc.vector.tensor_scalar_mul(t_[:], src, 4097.0)
            a_ = small.tile(list(shape), f32, name=f"{pref}_a")
            tt(a_[:], t_[:], src, SUB)
            hi = small.tile(list(shape), f32, name=f"{pref}_hi")
            tt(hi[:], t_[:], a_[:], SUB)
            lo = small.tile(list(shape), f32, name=f"{pref}_lo")
            tt(lo[:], src, hi[:], SUB)
            return hi, lo

        dh, dl = vsplit(d2[:], "dsp")

        def resid(qap, out_name, shape, dhb, dlb, db):
            """exact q*d - 224 via Dekker two-product (f32 ops only)"""
            p_ = small.tile(list(shape), f32, name=f"{out_name}_p")
            tt(p_[:], qap, db, MUL)
            qh, ql = vsplit(qap, f"{out_name}_qs", shape)
            w = small.tile(list(shape), f32, name=f"{out_name}_w")
            tt(w[:], qh[:], dhb, MUL)
            tt(w[:], w[:], p_[:], SUB)
            w2 = small.tile(list(shape), f32, name=f"{out_name}_w2")
            tt(w2[:], qh[:], dlb, MUL)
            tt(w[:], w[:], w2[:], ADD)
            tt(w2[:], ql[:], dhb, MUL)
            tt(w[:], w[:], w2[:], ADD)
            tt(w2[:], ql[:], dlb, MUL)
            tt(w[:], w[:], w2[:], ADD)
            nc.vector.tensor_scalar_sub(p_[:], p_[:], FP8_HW_MAX)
            R_ = small.tile(list(shape), f32, name=f"{out_name}_R")
            tt(R_[:], p_[:], w[:], ADD)
            return R_

        r2 = small.tile([1, 2], f32, name="r2")
        nc.vector.reciprocal(r2[:], d2[:])
        y0 = small.tile([1, 2], f32, name="y0")
        nc.vector.tensor_scalar_mul(y0[:], r2[:], FP8_HW_MAX)
        R0 = resid(y0[:], "n0", (1, 2), dh[:], dl[:], d2[:])
        corr = small.tile([1, 2], f32, name="corr")
        tt(corr[:], R0[:], r2[:], MUL)
        y = small.tile([1, 2], f32, name="yref")
        tt(y[:], y0[:], corr[:], SUB)

        # ulp(y) from the exponent bits; 5 candidates covering both binade sides
        um = small.tile([1, 2], f32, name="um")
        nc.vector.tensor_scalar(
            um[:].bitcast(u32), y[:].bitcast(u32), 0x7F800000, None,
            mybir.AluOpType.bitwise_and,
        )
        ul = small.tile([1, 2], f32, name="ul")
        nc.vector.tensor_scalar_mul(ul[:], um[:], 2.0 ** -23)
        cand = c3("cand")
        nc.vector.tensor_copy(cand[:, :, 0:1], y[:, :, None])
        tt(cand[:, :, 1:2], y[:, :, None], ul[:, :, None], ADD)
        tt(cand[:, :, 2:3], y[:, :, None], ul[:, :, None], SUB)
        nc.vector.tensor_scalar_mul(cand[:, :, 3:4], y[:, :, None], 1.0 - 2.0 ** -24)
        nc.vector.tensor_scalar_mul(cand[:, :, 4:5], y[:, :, None], 1.0 + 2.0 ** -24)

        dhb = dh[:, :, None].to_broadcast((1, 2, NCAND))
        dlb = dl[:, :, None].to_broadcast((1, 2, NCAND))
        db = d2[:, :, None].to_broadcast((1, 2, NCAND))
        Rc = resid(cand[:], "cc", (1, 2, NCAND), dhb, dlb, db)
        R2c = c3("R2c")
        tt(R2c[:], Rc[:], Rc[:], MUL)
        minr = small.tile([1, 2], f32, name="minr")
        nc.vector.tensor_reduce(minr[:], R2c[:], axis=X, op=mybir.AluOpType.min)
        mask = c3("mask")
        tt(mask[:], R2c[:], minr[:, :, None].to_broadcast((1, 2, NCAND)),
           mybir.AluOpType.is_equal)
        qm = c3("qm")
        tt(qm[:], cand[:], mask[:], MUL)
        s2 = small.tile([1, 2], f32, name="s2")
        nc.vector.tensor_reduce(s2[:], qm[:], axis=X, op=MAX)
        sk_b = bcast_scalar(s2[:, 0:1], "sk")
        sx_b = bcast_scalar(s2[:, 1:2], "sx")

        inv1 = small.tile([1, 1], f32, name="inv1")
        nc.vector.tensor_tensor(inv1[:], d2[:, 0:1], d2[:, 1:2], mybir.AluOpType.mult)
        nc.vector.tensor_scalar_mul(inv1[:], inv1[:], 1.0 / (FP8_HW_MAX * FP8_HW_MAX))
        inv_b = bcast_scalar(inv1[:], "inv")

        # bias shard, [P, nu]: bias_t[p, ub] = bias[ub*128 + p]
        bias_t = const.tile([P, nu], f32, name="bias_t")
        nc.sync.dma_start(bias_t[:], bsh.rearrange("(o p) -> p o", p=P))

        # ---- phase 2: quantize kernel shard, resident fp8 [P, ko_n, us] ----
        # Two-step quantize everywhere: an in-place f32 multiply (DVE, rounds
        # RNE32 exactly like the reference's x*scale) then a separate fp8
        # convert (ACT). The DVE's fused multiply+fp8-convert rounds the exact
        # product once, which disagrees with the reference's two roundings for
        # ~1e-6 of near-tie elements (~2e-3 absmax).
        kq = kqp.tile([P, ko_n, us], fp8, name="kq")
        for g in range(len(kret)):
            for j in range(KPRE_PER_GROUP):
                ko = n_stream + g * KPRE_PER_GROUP + j
                nc.vector.tensor_scalar_mul(kret[g][:, j], kret[g][:, j], sk_b[:])
                nc.scalar.copy(kq[:, ko], kret[g][:, j])
        last_restream_dma = last_xscan_dma
        for ko in range(n_stream):
            st = kstage.tile([P, us], f32, tag="kst", name="kq_st")
            dma = ldq(ko).dma_start(st[:], ksh[ko * P : (ko + 1) * P, :])
            if ko == 0 and last_xscan_dma is not None:
                add_dep_helper(
                    dma.ins, last_xscan_dma.ins, sync=True,
                    reason="kq re-stream starts after the scans",
                )
            last_restream_dma = dma
            nc.vector.tensor_scalar_mul(st[:], st[:], sk_b[:])
            nc.scalar.copy(kq[:, ko], st[:])

        # ---- phase 3: stream x blocks, fp8 DoubleRow matmuls, fused epilogue ----
        gelu = mybir.ActivationFunctionType.Gelu_apprx_tanh
        dr = mybir.MatmulPerfMode.DoubleRow
        t0 = 0
        for tb, blk in enumerate(blocks):
            tt_n = blk // nfree
            xq = xqp.tile([P, ko_n, blk], fp8, tag="xq", name="xq")
            for ko in range(ko_n):
                st = xstage.tile([P, xsmax], f32, tag="xst", name="xq_st")
                dma = nc.sync.dma_start(
                    st[:, :blk], xT[ko * P : (ko + 1) * P, t0 : t0 + blk]
                )
                if ko == 0 and last_restream_dma is not None:
                    # chain block streams so later blocks never jump the queue
                    add_dep_helper(
                        dma.ins, last_restream_dma.ins, sync=True,
                        reason="x blocks stream in consumption order",
                    )
                nc.vector.tensor_scalar_mul(st[:, :blk], st[:, :blk], sx_b[:])
                nc.any.tensor_copy(xq[:, ko], st[:, :blk])
            last_restream_dma = dma
            for ub in range(nu):
                pts = [
                    psum.tile([P, nfree], f32, tag="ps", name=f"ps{ti}")
                    for ti in range(tt_n)
                ]
                for kk in range(kk_n):
                    lw = kq[:, 2 * kk : 2 * kk + 2, ub * P : (ub + 1) * P]
                    for ti in range(tt_n):
                        nc.tensor.matmul(
                            pts[ti][:],
                            lw,
                            xq[:, 2 * kk : 2 * kk + 2, ti * nfree : (ti + 1) * nfree],
                            start=(kk == 0),
                            stop=(kk == kk_n - 1),
                            perf_mode=dr,
                        )
                for ti in range(tt_n):
                    ot = outp.tile([P, nfree], f32, tag="ot", name="ot")
                    nc.scalar.activation(
                        ot[:], pts[ti][:], gelu,
                        bias=bias_t[:, ub : ub + 1], scale=inv_b[:],
                    )
                    c0 = t0 + ti * nfree
                    nc.sync.dma_start(out[ub * P : (ub + 1) * P, c0 : c0 + nfree], ot[:])
            t0 += blk

    nc.compile()
    return nc


def make_in_maps(x, kern, bias, n_cores=N_CORES):
    tokens, d_in = x.shape
    us = kern.shape[1] // n_cores
    amx_t = tokens // n_cores
    xT = np.ascontiguousarray(x.T)
    in_maps = []
    for c in range(n_cores):
        in_maps.append(
            {
                "xT": xT,
                "xsl": np.ascontiguousarray(xT[:, c * amx_t : (c + 1) * amx_t]),
                "ksh": np.ascontiguousarray(kern[:, c * us : (c + 1) * us]),
                "bsh": np.ascontiguousarray(bias[c * us : (c + 1) * us]),
            }
        )
    return in_maps


_CACHE = {}


def _built():
    if "nc" not in _CACHE:
        _CACHE["nc"] = build()
    return _CACHE["nc"]


def run(x, kern, bias, trace=False, **kwargs):
    """Run on hardware; returns (full_output, BassKernelResults)."""
    nc = _built()
    in_maps = make_in_maps(x, kern, bias)
    res = run_bass_kernel_spmd(
        nc, in_maps, core_ids=list(range(N_CORES)), trace=trace, **kwargs
    )
    shards = [res.results[c]["out"] for c in range(N_CORES)]
    full = np.concatenate(shards, axis=0)  # [units, tokens]
    return full.T, res


def kernel(x, kernel, bias):
    out, _ = run(
        np.ascontiguousarray(x, dtype=np.float32),
        np.ascontiguousarray(kernel, dtype=np.float32),
        np.ascontiguousarray(bias, dtype=np.float32),
    )
    return out



# revision 6
# speedup vs baseline: 1.0376x; 1.0376x over previous
"""Trainium2 Bass kernel: fp8-quantized Dense (8192x4096 @ 4096x16384) + bias + tanh-GELU.

Strategy (tensor-parallel over units, 8 cores), v2:
  - host: transpose x -> xT [d_in, tokens]; shard kernel/bias along units.
  - device per core:
      phase 1: amax scans (k shard 32 MiB, then this core's 1/8 token slice of
               xT 16 MiB) in 1 MiB chunks on both HWDGE rings; one
               AllReduce(max) carries [amax_k, amax_x]. CC input/readback DMAs
               ride the gpsimd SWDGE queue so they never block the rings.
      phase 2: scales via reciprocal + one Newton step (~1 ulp; the exact-RNE
               division of v1 isn't needed at the 2e-2 gate). Fused one-op
               quantizes: k on DVE (tensor_scalar mul -> fp8), x on ACT
               (activation Copy w/ scale -> fp8).
      phase 3: k shard restreamed as [128,2,1024] column-half chunks, units
               0..1023 first: block 0 runs kk-ordered in two 8-ub PSUM groups
               so its matmuls consume chunks in exactly the restream order --
               the PE starts ~195 us in and the restream hides behind it.
               x streams in 512-token blocks (f32, quantized on arrival into
               a double-buffered fp8 xq), ub-ordered DoubleRow matmuls,
               ACT epilogue gelu_tanh(psum * inv_scale + bias) -> f16 out.
  - fp8 numerics: quantize with 224/amax (half the reference's 448/amax, a
    power-of-two ratio) so the TRN fp8e4 grid matches OCP e4m3fn exactly in
    [-240, 240]; dequant amax_x*amax_k/224^2 restores the reference values.
  - output is produced transposed ([units, tokens] f16 per core); the host
    gathers shards, transposes, and upcasts to f32.
"""

import sys

sys.path.insert(0, "/opt/trn_rl_repo")

from contextlib import ExitStack

import numpy as np

import concourse.bacc as bacc
import concourse.tile as tile
from concourse import mybir
from concourse.bass_utils import run_bass_kernel_spmd

P = 128
FP8_HW_MAX = 224.0  # 448/2: keeps hw fp8 values inside TRN's +/-240 range

TOKENS, D_IN, UNITS, N_CORES = 8192, 4096, 16384, 8
US = UNITS // N_CORES          # 2048 units per core
KO_N = D_IN // P               # 32 d_in slabs
NPAIR = KO_N // 2              # 16 DoubleRow (256-contraction) steps
NU = US // P                   # 16 output unit-blocks
BLK = 512                      # token block
NBLK = TOKENS // BLK           # 16
AMX_T = TOKENS // N_CORES      # 1024 tokens scanned per core


def build(n_cores=N_CORES):
    dt = mybir.dt
    f32 = dt.float32
    f16 = dt.float16
    fp8 = dt.float8e4
    X = mybir.AxisListType.X
    MAX = mybir.AluOpType.max
    COPY = mybir.ActivationFunctionType.Copy
    GELU = mybir.ActivationFunctionType.Gelu_apprx_tanh
    DR = mybir.MatmulPerfMode.DoubleRow

    nc = bacc.Bacc("TRN2", target_bir_lowering=False, debug=False, num_devices=n_cores)
    xT = nc.dram_tensor("xT", [D_IN, TOKENS], f32, kind="ExternalInput").ap()
    xsl = nc.dram_tensor("xsl", [D_IN, AMX_T], f32, kind="ExternalInput").ap()
    ksh = nc.dram_tensor("ksh", [D_IN, US], f32, kind="ExternalInput").ap()
    bsh = nc.dram_tensor("bsh", [US], f32, kind="ExternalInput").ap()
    out = nc.dram_tensor("out", [US, TOKENS], f16, kind="ExternalOutput").ap()

    xTr = xT.rearrange("(n p) t -> p n t", p=P)    # [128, 32, 8192]
    xslr = xsl.rearrange("(n p) t -> p n t", p=P)  # [128, 32, 1024]
    kshr = ksh.rearrange("(n p) c -> p n c", p=P)  # [128, 32, 2048]

    from concourse.tile_rust import add_dep_helper
    from concourse import bass_isa

    with tile.TileContext(nc) as tc, ExitStack() as ctx:
        const = ctx.enter_context(tc.tile_pool(name="const", bufs=1))
        small = ctx.enter_context(tc.tile_pool(name="small", bufs=1))
        kqp = ctx.enter_context(tc.tile_pool(name="kqp", bufs=1))      # 64 KiB/part
        kst = ctx.enter_context(tc.tile_pool(name="kst", bufs=4))      # 4x8 KiB
        xpool = ctx.enter_context(tc.tile_pool(name="xpool", bufs=8))  # 8x8 KiB
        xqp = ctx.enter_context(tc.tile_pool(name="xqp", bufs=2))      # 2x16 KiB
        outp = ctx.enter_context(tc.tile_pool(name="outp", bufs=8))    # 8x1 KiB
        psum = ctx.enter_context(tc.tile_pool(name="psum", bufs=8, space="PSUM"))
        dram = ctx.enter_context(tc.tile_pool(name="dram", bufs=1, space="DRAM"))

        def ring(i):
            return nc.sync if i % 2 == 0 else nc.scalar

        # ---- bias shard, [P, NU]: bias_t[p, ub] = bias[ub*128 + p] ----
        bias_t = const.tile([P, NU], f32, name="bias_t")
        nc.sync.dma_start(bias_t[:], bsh.rearrange("(o p) -> p o", p=P))

        # ---- phase 1a: k-shard amax scan, 32 x [P,2,1024] chunks ----
        rk = const.tile([P, KO_N], f32, name="rk")
        last_scan = None
        for i in range(KO_N):
            pr, h = i // 2, i % 2
            st = kst.tile([P, 2, 1024], f32, tag="kst", name="kscan")
            last_scan = ring(i).dma_start(
                st[:], kshr[:, 2 * pr : 2 * pr + 2, h * 1024 : (h + 1) * 1024]
            )
            nc.vector.tensor_reduce(
                rk[:, i : i + 1], st[:].rearrange("p a b -> p (a b)"), axis=X,
                op=MAX, apply_absolute_value=True,
            )
        last_kscan = last_scan

        # ---- phase 1b: x token-slice amax scan, 16 x [P,2,1024] chunks ----
        rx = const.tile([P, NPAIR], f32, name="rx")
        for i in range(NPAIR):
            st = xpool.tile([P, 2, 1024], f32, tag="xst", name="xscan")
            dma = ring(i).dma_start(st[:], xslr[:, 2 * i : 2 * i + 2, :])
            if i == 0:
                add_dep_helper(dma.ins, last_kscan.ins, sync=True,
                               reason="x-scan after k-scan")
            last_scan = dma
            nc.vector.tensor_reduce(
                rx[:, i : i + 1], st[:].rearrange("p a b -> p (a b)"), axis=X,
                op=MAX, apply_absolute_value=True,
            )
        last_xscan = last_scan

        # ---- single AllReduce(max) of [amax_k, amax_x] ----
        colk = small.tile([P, 1], f32, name="colk")
        nc.vector.tensor_reduce(colk[:], rk[:], axis=X, op=MAX)
        nc.gpsimd.partition_all_reduce(colk[:], colk[:], P, bass_isa.ReduceOp.max)
        colx = small.tile([P, 1], f32, name="colx")
        nc.vector.tensor_reduce(colx[:], rx[:], axis=X, op=MAX)
        nc.gpsimd.partition_all_reduce(colx[:], colx[:], P, bass_isa.ReduceOp.max)

        pk8 = small.tile([1, 8], f32, name="pk8")
        nc.vector.memset(pk8[:], 0.0)
        nc.vector.tensor_copy(pk8[:, 0:1], colk[0:1, :])
        nc.vector.tensor_copy(pk8[:, 1:2], colx[0:1, :])
        cc_in = dram.tile([1, 8], f32, name="cc_in")
        nc.gpsimd.dma_start(cc_in[:], pk8[:])
        cc_out = dram.tile([1, 8], f32, name="cc_out", addr_space="Shared")
        nc.gpsimd.collective_compute(
            "AllReduce", MAX,
            replica_groups=[list(range(n_cores))],
            ins=[cc_in[:].opt()], outs=[cc_out[:].opt()],
        )
        g8 = small.tile([1, 8], f32, name="g8")
        nc.gpsimd.dma_start(g8[:], cc_out[:])

        # ---- scales: s = 224/d via reciprocal + one Newton step ----
        MUL = mybir.AluOpType.mult
        SUB = mybir.AluOpType.subtract
        d2 = small.tile([1, 2], f32, name="d2")
        nc.vector.tensor_scalar_max(d2[:], g8[:, 0:2], 1e-12)
        r2 = small.tile([1, 2], f32, name="r2")
        nc.vector.reciprocal(r2[:], d2[:])
        y0 = small.tile([1, 2], f32, name="y0")
        nc.vector.tensor_scalar_mul(y0[:], r2[:], FP8_HW_MAX)
        t2 = small.tile([1, 2], f32, name="t2")
        nc.vector.tensor_tensor(t2[:], y0[:], d2[:], MUL)
        nc.vector.tensor_scalar_sub(t2[:], t2[:], FP8_HW_MAX)  # y0*d - 224
        nc.vector.tensor_tensor(t2[:], t2[:], r2[:], MUL)
        s2 = small.tile([1, 2], f32, name="s2")
        nc.vector.tensor_tensor(s2[:], y0[:], t2[:], SUB)      # y0 - r*(y0*d-224)

        def bcast(src11, name):
            b = const.tile([P, 1], f32, name=name)
            nc.gpsimd.partition_broadcast(b[:], src11)
            return b

        sk_b = bcast(s2[:, 0:1], "sk_b")
        sx_b = bcast(s2[:, 1:2], "sx_b")
        inv1 = small.tile([1, 1], f32, name="inv1")
        nc.vector.tensor_tensor(inv1[:], d2[:, 0:1], d2[:, 1:2], MUL)
        nc.vector.tensor_scalar_mul(inv1[:], inv1[:], 1.0 / (FP8_HW_MAX * FP8_HW_MAX))
        inv_b = bcast(inv1[:], "inv_b")

        # ---- resident fp8 kernel shard [P, 32, 2048] ----
        kq = kqp.tile([P, KO_N, US], fp8, name="kq")

        # ---- x stream + fused ACT quantize (f32 -> *sx -> fp8) ----
        xq_tiles = {}
        last_stream = {"d": last_xscan}

        def stream_quant(b):
            xq_t = xqp.tile([P, KO_N, BLK], fp8, tag="xq", name=f"xq{b}")
            t0 = b * BLK
            for g in range(8):
                st = xpool.tile([P, 4, BLK], f32, tag="xst", name=f"xst{b}_{g}")
                dma = nc.sync.dma_start(
                    st[:], xTr[:, 4 * g : 4 * g + 4, t0 : t0 + BLK]
                )
                if g == 0:
                    add_dep_helper(dma.ins, last_stream["d"].ins, sync=True,
                                   reason="x blocks stream in consumption order")
                nc.scalar.activation(
                    xq_t[:, 4 * g : 4 * g + 4, :], st[:], COPY, scale=sx_b[:]
                )
            last_stream["d"] = dma
            xq_tiles[b] = xq_t

        stream_quant(0)
        stream_quant(1)

        # ---- k restream + fused DVE quantize, column-half 0 then 1 ----
        # (ring A / sync, behind the block-0/1 streams; block 0's kk-ordered
        #  matmuls consume chunks in exactly this order)
        first_rs = None
        for h in range(2):
            for k in range(NPAIR):
                st = kst.tile([P, 2, 1024], f32, tag="kst", name=f"rs{h}_{k}")
                dma = nc.sync.dma_start(
                    st[:], kshr[:, 2 * k : 2 * k + 2, h * 1024 : (h + 1) * 1024]
                )
                if first_rs is None:
                    first_rs = dma
                    add_dep_helper(dma.ins, last_stream["d"].ins, sync=True,
                                   reason="restream behind block-0/1 streams")
                nc.vector.tensor_scalar_mul(
                    kq[:, 2 * k : 2 * k + 2, h * 1024 : (h + 1) * 1024],
                    st[:], sk_b[:],
                )
        last_stream["d"] = dma  # chain block-2 stream behind the restream

        # ---- matmuls + epilogue ----
        def epilogue(b, ub, pt):
            ot = outp.tile([P, BLK], f16, tag="ot", name="ot")
            nc.scalar.activation(
                ot[:], pt[:], GELU, bias=bias_t[:, ub : ub + 1], scale=inv_b[:]
            )
            nc.scalar.dma_start(
                out[ub * P : (ub + 1) * P, b * BLK : (b + 1) * BLK], ot[:]
            )

        for b in range(NBLK):
            xq_t = xq_tiles.pop(b)
            if b == 0:
                # kk-ordered, two 8-ub groups aligned with the restream halves
                for grp in range(2):
                    pts = [
                        psum.tile([P, BLK], f32, tag="ps", name=f"b0p{grp}_{j}")
                        for j in range(8)
                    ]
                    for kk in range(NPAIR):
                        for j in range(8):
                            ub = grp * 8 + j
                            nc.tensor.matmul(
                                pts[j][:],
                                kq[:, 2 * kk : 2 * kk + 2, ub * P : (ub + 1) * P],
                                xq_t[:, 2 * kk : 2 * kk + 2, :],
                                start=(kk == 0), stop=(kk == NPAIR - 1),
                                perf_mode=DR,
                            )
                    for j in range(8):
                        epilogue(b, grp * 8 + j, pts[j])
            else:
                for ub in range(NU):
                    pt = psum.tile([P, BLK], f32, tag="ps", name=f"ps{ub}")
                    for kk in range(NPAIR):
                        nc.tensor.matmul(
                            pt[:],
                            kq[:, 2 * kk : 2 * kk + 2, ub * P : (ub + 1) * P],
                            xq_t[:, 2 * kk : 2 * kk + 2, :],
                            start=(kk == 0), stop=(kk == NPAIR - 1),
                            perf_mode=DR,
                        )
                    epilogue(b, ub, pt)
            if b + 2 < NBLK:
                stream_quant(b + 2)

    nc.compile()
    return nc


def make_in_maps(x, kern, bias, n_cores=N_CORES):
    xT = np.ascontiguousarray(x.T)
    us = kern.shape[1] // n_cores
    amx_t = x.shape[0] // n_cores
    in_maps = []
    for c in range(n_cores):
        in_maps.append(
            {
                "xT": xT,
                "xsl": np.ascontiguousarray(xT[:, c * amx_t : (c + 1) * amx_t]),
                "ksh": np.ascontiguousarray(kern[:, c * us : (c + 1) * us]),
                "bsh": np.ascontiguousarray(bias[c * us : (c + 1) * us]),
            }
        )
    return in_maps


_CACHE = {}


def _built():
    if "nc" not in _CACHE:
        _CACHE["nc"] = build()
    return _CACHE["nc"]


def run(x, kern, bias, trace=False, **kwargs):
    """Run on hardware; returns (full_output, BassKernelResults)."""
    nc = _built()
    in_maps = make_in_maps(x, kern, bias)
    res = run_bass_kernel_spmd(
        nc, in_maps, core_ids=list(range(N_CORES)), trace=trace, **kwargs
    )
    shards = [res.results[c]["out"] for c in range(N_CORES)]
    full = np.concatenate(shards, axis=0)  # [units, tokens] f16
    return full.T.astype(np.float32), res


def kernel(x, kernel, bias):
    out, _ = run(
        np.ascontiguousarray(x, dtype=np.float32),
        np.ascontiguousarray(kernel, dtype=np.float32),
        np.ascontiguousarray(bias, dtype=np.float32),
    )
    return out


# revision 8
# speedup vs baseline: 1.0504x; 1.0123x over previous
"""Trainium2 Bass kernel: fp8-quantized Dense (8192x4096 @ 4096x16384) + bias + tanh-GELU.

Strategy (tensor-parallel over units, 8 cores), v2:
  - host: transpose x -> xT [d_in, tokens]; shard kernel/bias along units.
  - device per core:
      phase 1: amax scans (k shard 32 MiB, then this core's 1/8 token slice of
               xT 16 MiB) in 1 MiB chunks on both HWDGE rings; one
               AllReduce(max) carries [amax_k, amax_x]. CC input/readback DMAs
               ride the gpsimd SWDGE queue so they never block the rings.
      phase 2: scales via reciprocal + one Newton step (~1 ulp; the exact-RNE
               division of v1 isn't needed at the 2e-2 gate). Fused one-op
               quantizes: k on DVE (tensor_scalar mul -> fp8), x on ACT
               (activation Copy w/ scale -> fp8).
      phase 3: k shard restreamed as [128,2,1024] column-half chunks, units
               0..1023 first: block 0 runs kk-ordered in two 8-ub PSUM groups
               so its matmuls consume chunks in exactly the restream order --
               the PE starts ~195 us in and the restream hides behind it.
               x streams in 512-token blocks (f32, quantized on arrival into
               a double-buffered fp8 xq), ub-ordered DoubleRow matmuls,
               ACT epilogue gelu_tanh(psum * inv_scale + bias) -> f16 out.
  - fp8 numerics: quantize with 224/amax (half the reference's 448/amax, a
    power-of-two ratio) so the TRN fp8e4 grid matches OCP e4m3fn exactly in
    [-240, 240]; dequant amax_x*amax_k/224^2 restores the reference values.
  - output is produced transposed ([units, tokens] f16 per core); the host
    gathers shards, transposes, and upcasts to f32.
"""

import sys

sys.path.insert(0, "/opt/trn_rl_repo")

from contextlib import ExitStack

import numpy as np

import concourse.bacc as bacc
import concourse.tile as tile
from concourse import mybir
from concourse.bass_utils import run_bass_kernel_spmd

P = 128
FP8_HW_MAX = 224.0  # 448/2: keeps hw fp8 values inside TRN's +/-240 range

TOKENS, D_IN, UNITS, N_CORES = 8192, 4096, 16384, 8
US = UNITS // N_CORES          # 2048 units per core
KO_N = D_IN // P               # 32 d_in slabs
NPAIR = KO_N // 2              # 16 DoubleRow (256-contraction) steps
NU = US // P                   # 16 output unit-blocks
BLK = 512                      # token block
NBLK = TOKENS // BLK           # 16
AMX_T = TOKENS // N_CORES      # 1024 tokens scanned per core


def build(n_cores=N_CORES):
    dt = mybir.dt
    f32 = dt.float32
    f16 = dt.float16
    fp8 = dt.float8e4
    X = mybir.AxisListType.X
    MAX = mybir.AluOpType.max
    COPY = mybir.ActivationFunctionType.Copy
    GELU = mybir.ActivationFunctionType.Gelu_apprx_tanh
    DR = mybir.MatmulPerfMode.DoubleRow

    nc = bacc.Bacc("TRN2", target_bir_lowering=False, debug=False, num_devices=n_cores)
    xT = nc.dram_tensor("xT", [D_IN, TOKENS], f32, kind="ExternalInput").ap()
    xsl = nc.dram_tensor("xsl", [D_IN, AMX_T], f32, kind="ExternalInput").ap()
    ksh = nc.dram_tensor("ksh", [D_IN, US], f32, kind="ExternalInput").ap()
    bsh = nc.dram_tensor("bsh", [US], f32, kind="ExternalInput").ap()
    out = nc.dram_tensor("out", [US, TOKENS], f16, kind="ExternalOutput").ap()

    xTr = xT.rearrange("(n p) t -> p n t", p=P)    # [128, 32, 8192]
    xslr = xsl.rearrange("(n p) t -> p n t", p=P)  # [128, 32, 1024]
    kshr = ksh.rearrange("(n p) c -> p n c", p=P)  # [128, 32, 2048]

    from concourse.tile_rust import add_dep_helper
    from concourse import bass_isa

    with tile.TileContext(nc) as tc, ExitStack() as ctx:
        const = ctx.enter_context(tc.tile_pool(name="const", bufs=1))
        small = ctx.enter_context(tc.tile_pool(name="small", bufs=1))
        kqp = ctx.enter_context(tc.tile_pool(name="kqp", bufs=1))      # 64 KiB/part
        kst = ctx.enter_context(tc.tile_pool(name="kst", bufs=4))      # 4x8 KiB
        xpool = ctx.enter_context(tc.tile_pool(name="xpool", bufs=6))  # 6x8 KiB
        xqp = ctx.enter_context(tc.tile_pool(name="xqp", bufs=3))      # 3x16 KiB
        outp = ctx.enter_context(tc.tile_pool(name="outp", bufs=8))    # 8x1 KiB
        psum = ctx.enter_context(tc.tile_pool(name="psum", bufs=8, space="PSUM"))
        dram = ctx.enter_context(tc.tile_pool(name="dram", bufs=1, space="DRAM"))

        def ring(i):
            return nc.sync if i % 2 == 0 else nc.scalar

        # ---- bias shard, [P, NU]: bias_t[p, ub] = bias[ub*128 + p] ----
        bias_t = const.tile([P, NU], f32, name="bias_t")
        nc.sync.dma_start(bias_t[:], bsh.rearrange("(o p) -> p o", p=P))

        # ---- amax scans: 2 MiB chunks staged in the (idle) xq pool ----
        # k: 16 x [P,2,2048], x-slice: 8 x [P,4,1024], alternating HWDGE rings.
        rk = const.tile([P, NPAIR], f32, name="rk")
        last_scan = None
        for i in range(NPAIR):
            st = xqp.tile([P, 2, 2048], f32, tag="xq", name="kscan")
            last_scan = ring(i).dma_start(st[:], kshr[:, 2 * i : 2 * i + 2, :])
            nc.vector.tensor_reduce(
                rk[:, i : i + 1], st[:].rearrange("p a b -> p (a b)"), axis=X,
                op=MAX, apply_absolute_value=True,
            )
        last_kscan = last_scan

        rx = const.tile([P, 8], f32, name="rx")
        for i in range(8):
            st = xqp.tile([P, 4, 1024], f32, tag="xq", name="xscan")
            dma = ring(i).dma_start(st[:], xslr[:, 4 * i : 4 * i + 4, :])
            if i == 0:
                add_dep_helper(dma.ins, last_kscan.ins, sync=True,
                               reason="x-scan after k-scan")
            last_scan = dma
            nc.vector.tensor_reduce(
                rx[:, i : i + 1], st[:].rearrange("p a b -> p (a b)"), axis=X,
                op=MAX, apply_absolute_value=True,
            )
        last_xscan = last_scan

        # ---- AllGather of per-core [amax_k, amax_x]; local max-combine ----
        colk = small.tile([P, 1], f32, name="colk")
        nc.vector.tensor_reduce(colk[:], rk[:], axis=X, op=MAX)
        nc.gpsimd.partition_all_reduce(colk[:], colk[:], P, bass_isa.ReduceOp.max)
        colx = small.tile([P, 1], f32, name="colx")
        nc.vector.tensor_reduce(colx[:], rx[:], axis=X, op=MAX)
        nc.gpsimd.partition_all_reduce(colx[:], colx[:], P, bass_isa.ReduceOp.max)

        pk2 = small.tile([1, 2], f32, name="pk2")
        nc.vector.tensor_copy(pk2[:, 0:1], colk[0:1, :])
        nc.vector.tensor_copy(pk2[:, 1:2], colx[0:1, :])
        cc_in = dram.tile([1, 2], f32, name="cc_in")
        nc.gpsimd.dma_start(cc_in[:], pk2[:])
        cc_out = dram.tile([1, 2 * n_cores], f32, name="cc_out", addr_space="Shared")
        nc.gpsimd.collective_compute(
            "AllGather", mybir.AluOpType.bypass,
            replica_groups=[list(range(n_cores))],
            ins=[cc_in[:].opt()], outs=[cc_out[:].opt()],
        )
        g16 = small.tile([1, 2 * n_cores], f32, name="g16")
        nc.gpsimd.dma_start(g16[:], cc_out[:])

        # ---- scales: s = 224/d via reciprocal + one Newton step ----
        MUL = mybir.AluOpType.mult
        SUB = mybir.AluOpType.subtract
        d2 = small.tile([1, 2], f32, name="d2")
        nc.vector.tensor_copy(d2[:], g16[:, 0:2])
        for r in range(1, n_cores):
            nc.vector.tensor_tensor(d2[:], d2[:], g16[:, 2 * r : 2 * r + 2], MAX)
        nc.vector.tensor_scalar_max(d2[:], d2[:], 1e-12)
        r2 = small.tile([1, 2], f32, name="r2")
        nc.vector.reciprocal(r2[:], d2[:])
        y0 = small.tile([1, 2], f32, name="y0")
        nc.vector.tensor_scalar_mul(y0[:], r2[:], FP8_HW_MAX)
        t2 = small.tile([1, 2], f32, name="t2")
        nc.vector.tensor_tensor(t2[:], y0[:], d2[:], MUL)
        nc.vector.tensor_scalar_sub(t2[:], t2[:], FP8_HW_MAX)  # y0*d - 224
        nc.vector.tensor_tensor(t2[:], t2[:], r2[:], MUL)
        s2 = small.tile([1, 2], f32, name="s2")
        nc.vector.tensor_tensor(s2[:], y0[:], t2[:], SUB)      # y0 - r*(y0*d-224)

        def bcast(src11, name):
            b = const.tile([P, 1], f32, name=name)
            nc.gpsimd.partition_broadcast(b[:], src11)
            return b

        sk_b = bcast(s2[:, 0:1], "sk_b")
        sx_b = bcast(s2[:, 1:2], "sx_b")
        inv1 = small.tile([1, 1], f32, name="inv1")
        nc.vector.tensor_tensor(inv1[:], d2[:, 0:1], d2[:, 1:2], MUL)
        nc.vector.tensor_scalar_mul(inv1[:], inv1[:], 1.0 / (FP8_HW_MAX * FP8_HW_MAX))
        inv_b = bcast(inv1[:], "inv_b")

        # ---- resident fp8 kernel shard [P, 32, 2048] ----
        kq = kqp.tile([P, KO_N, US], fp8, name="kq")

        # ---- x stream + fused ACT quantize (f32 -> *sx -> fp8) ----
        xq_tiles = {}
        last_stream = {"d": last_xscan}

        def stream_quant(b):
            xq_t = xqp.tile([P, KO_N, BLK], fp8, tag="xq", name=f"xq{b}")
            t0 = b * BLK
            for g in range(8):
                st = xpool.tile([P, 4, BLK], f32, tag="xst", name=f"xst{b}_{g}")
                dma = nc.sync.dma_start(
                    st[:], xTr[:, 4 * g : 4 * g + 4, t0 : t0 + BLK]
                )
                if g == 0:
                    add_dep_helper(dma.ins, last_stream["d"].ins, sync=True,
                                   reason="x blocks stream in consumption order")
                nc.scalar.activation(
                    xq_t[:, 4 * g : 4 * g + 4, :], st[:], COPY, scale=sx_b[:]
                )
            last_stream["d"] = dma
            xq_tiles[b] = xq_t

        stream_quant(0)

        # ---- k restream + fused DVE quantize, column-half 0 then 1 ----
        # (alternating rings: ring A behind block-0's stream, ring B behind
        #  the x-scan tail; block 0's kk-ordered matmuls consume chunks in
        #  exactly this order)
        chained = set()
        for h in range(2):
            for k in range(NPAIR):
                i = h * NPAIR + k
                st = kst.tile([P, 2, 1024], f32, tag="kst", name=f"rs{h}_{k}")
                dma = ring(i).dma_start(
                    st[:], kshr[:, 2 * k : 2 * k + 2, h * 1024 : (h + 1) * 1024]
                )
                if i % 2 not in chained:
                    chained.add(i % 2)
                    prev = last_stream["d"] if i % 2 == 0 else last_xscan
                    add_dep_helper(dma.ins, prev.ins, sync=True,
                                   reason="restream behind block-0 stream / x-scan")
                nc.vector.tensor_scalar_mul(
                    kq[:, 2 * k : 2 * k + 2, h * 1024 : (h + 1) * 1024],
                    st[:], sk_b[:],
                )
        last_stream["d"] = dma  # chain block-1 stream behind the restream
        stream_quant(1)

        # ---- matmuls + epilogue ----
        def epilogue(b, ub, pt):
            ot = outp.tile([P, BLK], f16, tag="ot", name="ot")
            nc.scalar.activation(
                ot[:], pt[:], GELU, bias=bias_t[:, ub : ub + 1], scale=inv_b[:]
            )
            nc.scalar.dma_start(
                out[ub * P : (ub + 1) * P, b * BLK : (b + 1) * BLK], ot[:]
            )

        for b in range(NBLK):
            xq_t = xq_tiles.pop(b)
            if b == 0:
                # kk-ordered, two 8-ub groups aligned with the restream halves
                for grp in range(2):
                    pts = [
                        psum.tile([P, BLK], f32, tag="ps", name=f"b0p{grp}_{j}")
                        for j in range(8)
                    ]
                    for kk in range(NPAIR):
                        for j in range(8):
                            ub = grp * 8 + j
                            nc.tensor.matmul(
                                pts[j][:],
                                kq[:, 2 * kk : 2 * kk + 2, ub * P : (ub + 1) * P],
                                xq_t[:, 2 * kk : 2 * kk + 2, :],
                                start=(kk == 0), stop=(kk == NPAIR - 1),
                                perf_mode=DR,
                            )
                    for j in range(8):
                        epilogue(b, grp * 8 + j, pts[j])
            else:
                for ub in range(NU):
                    pt = psum.tile([P, BLK], f32, tag="ps", name=f"ps{ub}")
                    for kk in range(NPAIR):
                        nc.tensor.matmul(
                            pt[:],
                            kq[:, 2 * kk : 2 * kk + 2, ub * P : (ub + 1) * P],
                            xq_t[:, 2 * kk : 2 * kk + 2, :],
                            start=(kk == 0), stop=(kk == NPAIR - 1),
                            perf_mode=DR,
                        )
                    epilogue(b, ub, pt)
            if b + 2 < NBLK:
                stream_quant(b + 2)

    nc.compile()
    return nc


def make_in_maps(x, kern, bias, n_cores=N_CORES):
    xT = np.ascontiguousarray(x.T)
    us = kern.shape[1] // n_cores
    amx_t = x.shape[0] // n_cores
    in_maps = []
    for c in range(n_cores):
        in_maps.append(
            {
                "xT": xT,
                "xsl": np.ascontiguousarray(xT[:, c * amx_t : (c + 1) * amx_t]),
                "ksh": np.ascontiguousarray(kern[:, c * us : (c + 1) * us]),
                "bsh": np.ascontiguousarray(bias[c * us : (c + 1) * us]),
            }
        )
    return in_maps


_CACHE = {}


def _built():
    if "nc" not in _CACHE:
        _CACHE["nc"] = build()
    return _CACHE["nc"]


def run(x, kern, bias, trace=False, **kwargs):
    """Run on hardware; returns (full_output, BassKernelResults)."""
    nc = _built()
    in_maps = make_in_maps(x, kern, bias)
    res = run_bass_kernel_spmd(
        nc, in_maps, core_ids=list(range(N_CORES)), trace=trace, **kwargs
    )
    shards = [res.results[c]["out"] for c in range(N_CORES)]
    full = np.concatenate(shards, axis=0)  # [units, tokens] f16
    return full.T.astype(np.float32), res


def kernel(x, kernel, bias):
    out, _ = run(
        np.ascontiguousarray(x, dtype=np.float32),
        np.ascontiguousarray(kernel, dtype=np.float32),
        np.ascontiguousarray(bias, dtype=np.float32),
    )
    return out
